# revision 9
# baseline (speedup 1.0000x reference)
"""RGAT (2-layer) + portfolio head for nn_GrossModel_20151986553247 on 8
Trainium2 NeuronCores via Bass.

Sharding: each core owns a contiguous slice of N/8=625 destination nodes for
all 12 graphs. Per graph-layer each core builds its slice of the
(node,relation)-major feature table [xr | s_k] with PE matmuls, an AllGather
shares the full table, and the edge phase runs as [125,1]-indexed indirect
DMA gathers + dense per-tile attention on DVE/ACT. The final portfolio layer
runs replicated after an AllReduce of the per-node predictions.

All floating-point model math runs on device; the host only computes integer
index/layout metadata (degree sort, padded slot tables) and reshapes weights.
"""
import os
import sys

sys.path.insert(0, "/opt/trn_rl_repo")
sys.path.insert(0, os.path.dirname(os.path.abspath(__file__)))

import numpy as np

S, N, F, R, E = 12, 5000, 64, 5, 256000
O1, O2 = 15, 10
BN_EPS = 1e-5
NCORES = 8
NSLICE = N // NCORES          # 625 nodes per core
NT = 5                        # dst tiles per core
TP = NSLICE // NT             # 125 nodes per tile
TBLK = NSLICE * R + 3         # per-core table rows (3125) + 3 zero pad rows
PAD_ROW = TBLK - 3            # row 3125 of block 0 is the global zero row
YB = NSLICE + 3               # 628
YROWS = S * NCORES * YB

last_results = None  # BassKernelResults of the most recent run (for timing)
last_nc = None
last_in_maps = None


def _prep(edge_index, edge_attr, edge_type):
    """Host-side integer/layout preprocessing. Returns per-core blobs."""
    percore = []
    for c in range(NCORES):
        lo, hi = c * NSLICE, (c + 1) * NSLICE
        g_idx, g_ea, g_et, g_sqi, g_ysi, tiles = [], [], [], [], [], []
        for g in range(S):
            src = edge_index[g, 0].astype(np.int64)
            dst = edge_index[g, 1].astype(np.int64)
            et = edge_type[g].astype(np.int64)
            ea = edge_attr[g, :, 0].astype(np.float32)
            m = (dst >= lo) & (dst < hi)
            src_l, dst_l, et_l, ea_l = src[m], dst[m] - lo, et[m], ea[m]
            deg = np.bincount(dst_l, minlength=NSLICE)
            order = np.argsort(-deg, kind="stable").astype(np.int64)
            ed_order = np.argsort(dst_l, kind="stable")
            starts = np.zeros(NSLICE + 1, np.int64)
            np.cumsum(deg, out=starts[1:])
            src_s, et_s, ea_s = src_l[ed_order], et_l[ed_order], ea_l[ed_order]
            gtiles = []
            for t in range(NT):
                rows = order[t * TP : (t + 1) * TP]
                dmax = max(int(deg[rows].max()), 1)
                idx = np.full((TP, dmax), PAD_ROW, np.int32)
                eav = np.zeros((TP, dmax), np.float32)
                etf = np.full((TP, dmax), float(R), np.float32)
                for p, j in enumerate(rows):
                    d = int(deg[j])
                    if d == 0:
                        continue
                    sl = slice(starts[j], starts[j] + d)
                    sj = src_s[sl]
                    idx[p, :d] = (
                        (sj // NSLICE) * TBLK + (sj % NSLICE) * R + et_s[sl]
                    ).astype(np.int32)
                    eav[p, :d] = ea_s[sl]
                    etf[p, :d] = et_s[sl].astype(np.float32)
                g_idx.append(idx)
                g_ea.append(eav)
                g_et.append(etf)
                g_sqi.append(rows.astype(np.int32).reshape(TP, 1))
                g_ysi.append(
                    (g * NCORES * YB + c * YB + rows).astype(np.int32).reshape(TP, 1)
                )
                gtiles.append(dmax)
            tiles.append(gtiles)
        percore.append(
            dict(
                idx=g_idx, ea=g_ea, etf=g_et,
                sqi=np.concatenate(g_sqi, axis=1),
                ysi=np.concatenate(g_ysi, axis=1),
                tiles=tiles,
            )
        )
    return percore


def _build_and_run(inputs):
    global last_results, last_nc, last_in_maps
    import tile_patch

    tile_patch.apply()
    import concourse.bass as bass
    import concourse.tile as tile
    import concourse.mybir as mybir
    from concourse.masks import make_identity
    from concourse.bass_utils import run_bass_kernel_spmd

    dt = mybir.dt
    Alu = mybir.AluOpType
    Act = mybir.ActivationFunctionType

    percore = _prep(
        np.asarray(inputs["edge_index"]),
        np.asarray(inputs["edge_attr"], np.float32),
        np.asarray(inputs["edge_type"]),
    )
    # SPMD: identical program on all cores -> per-(g,t) slot width is the max
    # across cores.
    tiles = [
        [max(percore[c]["tiles"][g][t] for c in range(NCORES)) for t in range(NT)]
        for g in range(S)
    ]
    totd = sum(sum(tg) for tg in tiles)

    def repack(pc):
        idx = np.full((TP, totd), PAD_ROW, np.int32)
        eav = np.zeros((TP, totd), np.float32)
        etf = np.full((TP, totd), float(R), np.float32)
        o = 0
        k = 0
        for g in range(S):
            for t in range(NT):
                d_c = pc["tiles"][g][t]
                d = tiles[g][t]
                idx[:, o : o + d_c] = pc["idx"][k]
                eav[:, o : o + d_c] = pc["ea"][k]
                etf[:, o : o + d_c] = pc["etf"][k]
                o += d
                k += 1
        return idx, eav, etf

    w1 = np.asarray(inputs["w1"], np.float32)   # [R, F, O1]
    w2 = np.asarray(inputs["w2"], np.float32)   # [R, O1, O2]
    wall1 = np.concatenate([w1[r] for r in range(R)], axis=1)          # [64, 75]
    wall2 = np.concatenate([w2[r] for r in range(R)], axis=1)          # [15, 50]
    w1T = np.ascontiguousarray(np.transpose(w1, (0, 2, 1)))            # [5,15,64]
    w2T = np.ascontiguousarray(np.transpose(w2, (0, 2, 1)))            # [5,10,15]
    xT = np.ascontiguousarray(
        np.transpose(np.asarray(inputs["x"], np.float32), (0, 2, 1))
    )  # [S, 64, N]
    fr = np.asarray(inputs["future_ret"], np.float32)

    aux = np.zeros((1, 160), np.float32)
    off = {}
    o = 0
    for nm in ["b1", "bn1_w", "bn1_b", "bn1_m", "bn1_v", "b2", "bn2_w", "bn2_b",
               "bn2_m", "bn2_v", "lin_w", "lin_b", "gamma"]:
        a = np.asarray(inputs[nm], np.float32).ravel()
        aux[0, o : o + a.size] = a
        off[nm] = o
        o += a.size

    nc = bass.Bass()
    T = dt.float32

    d_xT = nc.dram_tensor("xT", [S, F, NSLICE], T, kind="ExternalInput")
    d_fr = nc.dram_tensor("fr", [S, N], T, kind="ExternalInput")
    d_wall1 = nc.dram_tensor("wall1", [F, R * O1], T, kind="ExternalInput")
    d_wall2 = nc.dram_tensor("wall2", [O1, R * O2], T, kind="ExternalInput")
    d_w1T = nc.dram_tensor("w1T", [R, O1, F], T, kind="ExternalInput")
    d_w2T = nc.dram_tensor("w2T", [R, O2, O1], T, kind="ExternalInput")
    d_kq1 = nc.dram_tensor("kq1", [O1, 4], T, kind="ExternalInput")
    d_kq2 = nc.dram_tensor("kq2", [O2, 4], T, kind="ExternalInput")
    d_aux = nc.dram_tensor("aux", [1, 160], T, kind="ExternalInput")
    d_idx = nc.dram_tensor("idx", [TP, totd], dt.int32, kind="ExternalInput")
    d_ea = nc.dram_tensor("ea", [TP, totd], T, kind="ExternalInput")
    d_etf = nc.dram_tensor("etf", [TP, totd], T, kind="ExternalInput")
    d_sqi = nc.dram_tensor("sqi", [TP, S * NT], dt.int32, kind="ExternalInput")
    d_ysi = nc.dram_tensor("ysi", [TP, S * NT], dt.int32, kind="ExternalInput")
    d_w = nc.dram_tensor("w", [1, N], T, kind="ExternalOutput")

    d_Tloc = [nc.dram_tensor(f"Tloc{i}", [TBLK, 16], T) for i in range(2)]
    d_Tfull = [
        nc.dram_tensor(f"Tfull{i}", [NCORES * TBLK, 16], T, addr_space="Shared")
        for i in range(2)
    ]
    d_sq = [nc.dram_tensor(f"sq{i}", [NSLICE, 8], T) for i in range(2)]
    d_h = [nc.dram_tensor(f"h{i}", [NSLICE, 16], T) for i in range(2)]
    d_cc = nc.dram_tensor("ccd", [1, 2], T)
    d_combo = nc.dram_tensor("combod", [1, 64], T)
    d_y16 = nc.dram_tensor("y16", [YROWS, 4], T)
    d_y16f = nc.dram_tensor("y16f", [YROWS, 4], T, addr_space="Shared")

    IOO = bass.IndirectOffsetOnAxis

    def bcast_part(ap, parts):
        """View a [1, w] AP as [parts, w] via partition step 0 (DMA only)."""
        return bass.AP(ap.tensor, ap.offset, [[0, parts]] + [list(d) for d in ap.ap[1:]])

    def rawap(ap, extra_off, dims):
        return bass.AP(ap.tensor, ap.offset + extra_off, dims)

    with tile.TileContext(nc) as tc:
        with (
            tc.tile_pool(name="const", bufs=1) as constp,
            tc.tile_pool(name="xtp", bufs=2) as xtp,
            tc.tile_pool(name="psum", bufs=2, space="PSUM") as psum,
            tc.tile_pool(name="stage", bufs=4) as stagep,
            tc.tile_pool(name="idxp", bufs=3) as idxp,
            tc.tile_pool(name="gath", bufs=3) as gathp,
            tc.tile_pool(name="work", bufs=4) as workp,
            tc.tile_pool(name="small", bufs=6) as smallp,
            tc.tile_pool(name="hpool", bufs=2) as hpool,
        ):
            ident = constp.tile([128, 128], T)
            make_identity(nc, ident[:])
            aux_t = constp.tile([1, 160], T)
            nc.sync.dma_start(aux_t[:], d_aux[:])
            zero3 = constp.tile([3, 16], T)
            nc.vector.memset(zero3[:], 0.0)
            zbig = constp.tile([128, YROWS * 4 // 128], T)
            nc.vector.memset(zbig[:], 0.0)
            nc.sync.dma_start(
                d_y16[:].rearrange("(p a) c -> p a c", p=128),
                zbig[:].rearrange("p (a c) -> p a c", c=4),
            )

            rhs1 = constp.tile([F, 96], T)
            nc.sync.dma_start(rhs1[:, : R * O1], d_wall1[:])
            rhs2 = constp.tile([O1, 64], T)
            nc.sync.dma_start(rhs2[:, : R * O2], d_wall2[:])
            w1T_t = [constp.tile([O1, F], T, tag=f"w1T{r}", name=f"w1T{r}") for r in range(R)]
            w2T_t = [constp.tile([O2, O1], T, tag=f"w2T{r}", name=f"w2T{r}") for r in range(R)]
            for r in range(R):
                nc.sync.dma_start(w1T_t[r][:], d_w1T[r, :, :])
                nc.sync.dma_start(w2T_t[r][:], d_w2T[r, :, :])
            kq1 = constp.tile([O1, 4], T)
            nc.sync.dma_start(kq1[:], d_kq1[:])
            kq2 = constp.tile([O2, 4], T)
            nc.sync.dma_start(kq2[:], d_kq2[:])

            for r in range(R):
                pt1 = psum.tile([F, 2], T, space="PSUM", tag="pinit")
                nc.tensor.matmul(
                    pt1[:, 0:1], w1T_t[r][:], kq1[:, 0:1],
                    start=True, stop=True,
                )
                nc.tensor.matmul(
                    pt1[:, 1:2], w1T_t[r][:], kq1[:, 1:2],
                    start=True, stop=True,
                )
                nc.vector.tensor_copy(rhs1[:, 75 + r : 76 + r], pt1[:, 0:1])
                nc.vector.tensor_copy(rhs1[:, 80 + r : 81 + r], pt1[:, 1:2])
                pt2 = psum.tile([O1, 2], T, space="PSUM", tag="pinit")
                nc.tensor.matmul(
                    pt2[:, 0:1], w2T_t[r][:], kq2[:, 0:1],
                    start=True, stop=True,
                )
                nc.tensor.matmul(
                    pt2[:, 1:2], w2T_t[r][:], kq2[:, 1:2],
                    start=True, stop=True,
                )
                nc.vector.tensor_copy(rhs2[:, 50 + r : 51 + r], pt2[:, 0:1])
                nc.vector.tensor_copy(rhs2[:, 55 + r : 56 + r], pt2[:, 1:2])

            # c1 = lew1 @ e1, c2 = lew2 @ e2 (scalars) -> DRAM -> broadcast
            ccs = smallp.tile([1, 2], T, tag="ccs")
            pc1 = psum.tile([1, 1], T, space="PSUM", tag="pinit")
            nc.tensor.matmul(pc1[:], kq1[:, 3:4], kq1[:, 2:3], start=True, stop=True)
            nc.vector.tensor_copy(ccs[:, 0:1], pc1[:])
            pc2 = psum.tile([1, 1], T, space="PSUM", tag="pinit")
            nc.tensor.matmul(pc2[:], kq2[:, 3:4], kq2[:, 2:3], start=True, stop=True)
            nc.vector.tensor_copy(ccs[:, 1:2], pc2[:])
            nc.sync.dma_start(d_cc[:], ccs[:])
            cc_bc = constp.tile([TP, 2], T)
            nc.sync.dma_start(cc_bc[:], bcast_part(d_cc[:], TP))

            # BN combos on partition 0, then DRAM-broadcast to TP partitions
            combo = smallp.tile([1, 64], T, tag="combo")
            for nb, nw, nbb, nm_, nv, oc, osz in [
                ("b1", "bn1_w", "bn1_b", "bn1_m", "bn1_v", 0, O1),
                ("b2", "bn2_w", "bn2_b", "bn2_m", "bn2_v", 32, O2),
            ]:
                tmpc = smallp.tile([1, 16], T, tag="tmpc")
                nc.vector.tensor_scalar(
                    tmpc[:, :osz], aux_t[:, off[nv] : off[nv] + osz], BN_EPS, None,
                    op0=Alu.add,
                )
                nc.scalar.activation(tmpc[:, :osz], tmpc[:, :osz], Act.Sqrt)
                nc.vector.reciprocal(tmpc[:, :osz], tmpc[:, :osz])
                nc.vector.tensor_tensor(
                    combo[:, oc : oc + osz], tmpc[:, :osz],
                    aux_t[:, off[nw] : off[nw] + osz], op=Alu.mult,
                )
                t2 = smallp.tile([1, 16], T, tag="t2c")
                nc.vector.tensor_tensor(
                    t2[:, :osz], aux_t[:, off[nm_] : off[nm_] + osz],
                    aux_t[:, off[nb] : off[nb] + osz], op=Alu.subtract,
                )
                nc.vector.tensor_tensor(
                    t2[:, :osz], t2[:, :osz], combo[:, oc : oc + osz], op=Alu.mult
                )
                nc.vector.tensor_tensor(
                    combo[:, oc + 16 : oc + 16 + osz],
                    aux_t[:, off[nbb] : off[nbb] + osz], t2[:, :osz],
                    op=Alu.subtract,
                )
            nc.sync.dma_start(d_combo[:], combo[:])
            combo_bc = constp.tile([TP, 64], T)
            nc.sync.dma_start(combo_bc[:], bcast_part(d_combo[:], TP))
            aux_bc = constp.tile([TP, 160], T)
            nc.sync.dma_start(aux_bc[:], bcast_part(d_aux[:], TP))

            for i in range(2):
                nc.sync.dma_start(d_Tloc[i][PAD_ROW : PAD_ROW + 3, :], zero3[:])

            def build_table(g, layer, xsrc):
                K = F if layer == 0 else O1
                rhs = rhs1 if layer == 0 else rhs2
                nch = O1 if layer == 0 else O2
                wkoff = R * nch
                dT, dS = d_Tloc[g % 2], d_sq[g % 2]
                for t in range(NT):
                    pt = psum.tile([TP, 96], T, space="PSUM", tag="ptb")
                    nc.tensor.matmul(
                        pt[:, : wkoff + 2 * R],
                        xsrc[:K, t * TP : (t + 1) * TP],
                        rhs[:, : wkoff + 2 * R],
                        start=True, stop=True,
                    )
                    st = stagep.tile([TP, R * 16], T, tag="tstage")
                    xv = pt[:, :wkoff].rearrange("p (r c) -> p r c", r=R)
                    sv = st[:].rearrange("p (r c) -> p r c", r=R)
                    nc.vector.tensor_copy(sv[:, :, :nch], xv)
                    ptA = pt[:, wkoff : wkoff + R]
                    src3 = bass.AP(
                        ptA.tensor, ptA.offset,
                        [list(ptA.ap[0]), list(ptA.ap[1]), [0, 1]],
                    )
                    nc.vector.tensor_copy(sv[:, :, nch : nch + 1], src3)
                    sq = stagep.tile([TP, 8], T, tag="sqstage")
                    nc.vector.tensor_copy(sq[:, :R], pt[:, wkoff + R : wkoff + 2 * R])
                    nc.sync.dma_start(
                        dT[t * TP * R : (t + 1) * TP * R, :].rearrange(
                            "(p r) c -> p r c", r=R
                        ),
                        st[:].rearrange("p (r c) -> p r c", r=R),
                    )
                    nc.sync.dma_start(dS[t * TP : (t + 1) * TP, :], sq[:])

            def allgather_T(g, layer):
                with tc.tile_critical():
                    sem = nc.alloc_semaphore(f"ag{g}_{layer}")
                    nc.gpsimd.collective_compute(
                        "AllGather", mybir.AluOpType.bypass,
                        ins=[d_Tloc[g % 2][:]], outs=[d_Tfull[g % 2][:]],
                        replica_groups=[list(range(NCORES))],
                    ).then_inc(sem, 1)
                    nc.gpsimd.wait_ge(sem, 1)

            def edge_phase(g, layer, col0):
                nch = O1 if layer == 0 else O2
                oc = 0 if layer == 0 else 32
                outs = []
                o = col0
                for t in range(NT):
                    D = tiles[g][t]
                    it = idxp.tile([TP, D], dt.int32, tag="idx")
                    nc.sync.dma_start(it[:], d_idx[:, o : o + D])
                    eat = idxp.tile([TP, D], T, tag="ea")
                    nc.sync.dma_start(eat[:], d_ea[:, o : o + D])
                    ett = idxp.tile([TP, D], T, tag="etf")
                    nc.sync.dma_start(ett[:], d_etf[:, o : o + D])
                    sit = idxp.tile([TP, 1], dt.int32, tag="sqi")
                    nc.sync.dma_start(sit[:], d_sqi[:, g * NT + t : g * NT + t + 1])
                    sqt = smallp.tile([TP, 8], T, tag="sq")
                    nc.gpsimd.indirect_dma_start(
                        out=sqt[:], out_offset=None, in_=d_sq[g % 2][:],
                        in_offset=IOO(ap=sit[:], axis=0),
                    )
                    gt = gathp.tile([TP, D * 16], T, tag="gath")
                    for j in range(D):
                        nc.gpsimd.indirect_dma_start(
                            out=gt[:, j * 16 : (j + 1) * 16],
                            out_offset=None,
                            in_=d_Tfull[g % 2][:],
                            in_offset=IOO(ap=it[:, j : j + 1], axis=0),
                        )
                    gta = gt[:]
                    # alpha = c*ea + alpha_k + sq_sel
                    al = workp.tile([TP, D], T, tag="alpha")
                    nc.vector.tensor_scalar(
                        al[:], eat[:], cc_bc[:, layer : layer + 1], None, op0=Alu.mult
                    )
                    akv = rawap(gta, nch, [list(gta.ap[0]), [16, D]])
                    nc.vector.tensor_tensor(al[:], al[:], akv, op=Alu.add)
                    cmp = workp.tile([TP, D], T, tag="cmp")
                    sel = workp.tile([TP, D], T, tag="sel")
                    for r in range(R):
                        nc.vector.tensor_scalar(
                            cmp[:], ett[:], float(r), None, op0=Alu.is_equal
                        )
                        nc.vector.tensor_scalar(
                            sel[:], cmp[:], sqt[:, r : r + 1], None, op0=Alu.mult
                        )
                        nc.vector.tensor_tensor(al[:], al[:], sel[:], op=Alu.add)
                    nc.scalar.activation(al[:], al[:], Act.Lrelu, alpha=0.2)
                    nc.scalar.activation(al[:], al[:], Act.Exp)
                    nc.vector.tensor_scalar(
                        cmp[:], ett[:], float(R) - 0.5, None, op0=Alu.is_lt
                    )
                    nc.vector.tensor_tensor(al[:], al[:], cmp[:], op=Alu.mult)
                    den = smallp.tile([TP, 1], T, tag="den")
                    nc.vector.reduce_sum(den[:], al[:], axis=mybir.AxisListType.X)
                    nc.vector.tensor_scalar(den[:], den[:], 1e-16, None, op0=Alu.add)
                    nc.vector.reciprocal(den[:], den[:])
                    wm = gathp.tile([TP, D * nch], T, tag="wmsg")
                    wv = wm[:].rearrange("p (d c) -> p d c", d=D)
                    ala = al[:]
                    alb = rawap(ala, 0, [list(ala.ap[0]), list(ala.ap[1]), [0, nch]])
                    gvm = rawap(gta, 0, [list(gta.ap[0]), [16, D], [1, nch]])
                    nc.vector.tensor_tensor(wv, gvm, alb, op=Alu.mult)
                    agg = workp.tile([TP, 16], T, tag="agg")
                    wma = wm[:]
                    wvt = rawap(wma, 0, [list(wma.ap[0]), [1, nch], [nch, D]])
                    nc.vector.reduce_sum(agg[:, :nch], wvt, axis=mybir.AxisListType.X)
                    nc.vector.tensor_scalar(
                        agg[:, :nch], agg[:, :nch], den[:], None, op0=Alu.mult
                    )
                    nc.vector.tensor_tensor(
                        agg[:, :nch], agg[:, :nch], combo_bc[:, oc : oc + nch],
                        op=Alu.mult,
                    )
                    nc.vector.tensor_tensor(
                        agg[:, :nch], agg[:, :nch],
                        combo_bc[:, oc + 16 : oc + 16 + nch], op=Alu.add,
                    )
                    mn = workp.tile([TP, 16], T, tag="mn")
                    nc.vector.tensor_scalar(
                        mn[:, :nch], agg[:, :nch], 0.0, None, op0=Alu.min
                    )
                    nc.scalar.activation(mn[:, :nch], mn[:, :nch], Act.Exp)
                    nc.vector.tensor_scalar(
                        agg[:, :nch], agg[:, :nch], 0.0, None, op0=Alu.max
                    )
                    nc.vector.tensor_tensor(
                        agg[:, :nch], agg[:, :nch], mn[:, :nch], op=Alu.add
                    )
                    nc.vector.tensor_scalar(
                        agg[:, :nch], agg[:, :nch], -1.0, None, op0=Alu.add
                    )
                    if nch < 15:
                        nc.vector.memset(agg[:, nch:], 0.0)
                    outs.append((agg, sit))
                    o += D
                return outs

            col0 = 0
            for g in range(S):
                xt = xtp.tile([F, NSLICE], T, tag="xt")
                nc.sync.dma_start(xt[:], d_xT[g, :, :])
                build_table(g, 0, xt)
                allgather_T(g, 0)
                l1 = edge_phase(g, 0, col0)
                for t, (agg, sit) in enumerate(l1):
                    nc.gpsimd.indirect_dma_start(
                        out=d_h[g % 2][:], out_offset=IOO(ap=sit[:], axis=0),
                        in_=agg[:], in_offset=None,
                    )
                hT = hpool.tile([16, NSLICE], T, tag="hT")
                for t in range(NT):
                    hn = stagep.tile([TP, 16], T, tag="hn")
                    nc.sync.dma_start(hn[:], d_h[g % 2][t * TP : (t + 1) * TP, :])
                    ptr = psum.tile([16, TP], T, space="PSUM", tag="ptr")
                    nc.tensor.transpose(ptr[:], hn[:], ident[:TP, :TP])
                    nc.vector.tensor_copy(hT[:, t * TP : (t + 1) * TP], ptr[:])
                build_table(g, 1, hT)
                allgather_T(g, 1)
                l2 = edge_phase(g, 1, col0)
                for t, (agg, _) in enumerate(l2):
                    yt = smallp.tile([TP, O2], T, tag="yt")
                    nc.vector.tensor_tensor(
                        yt[:], agg[:, :O2],
                        aux_bc[:, off["lin_w"] : off["lin_w"] + O2], op=Alu.mult,
                    )
                    yst = stagep.tile([TP, 4], T, tag="yst")
                    nc.vector.memset(yst[:], 0.0)
                    nc.vector.reduce_sum(yst[:, 0:1], yt[:], axis=mybir.AxisListType.X)
                    nc.vector.tensor_scalar(
                        yst[:, 0:1], yst[:, 0:1],
                        aux_bc[:, off["lin_b"] : off["lin_b"] + 1], None, op0=Alu.add,
                    )
                    yit = idxp.tile([TP, 1], dt.int32, tag="ysi")
                    nc.sync.dma_start(yit[:], d_ysi[:, g * NT + t : g * NT + t + 1])
                    nc.gpsimd.indirect_dma_start(
                        out=d_y16[:], out_offset=IOO(ap=yit[:], axis=0),
                        in_=yst[:], in_offset=None,
                    )
                col0 += sum(tiles[g])

            with tc.tile_critical():
                sem = nc.alloc_semaphore("ar_y")
                nc.gpsimd.collective_compute(
                    "AllReduce", mybir.AluOpType.add,
                    ins=[d_y16[:]], outs=[d_y16f[:]],
                    replica_groups=[list(range(NCORES))],
                ).then_inc(sem, 1)
                nc.gpsimd.wait_ge(sem, 1)

            ysb = constp.tile([S, N], T)
            yf = d_y16f[:]
            ysrc = bass.AP(
                yf.tensor, 0,
                [[NCORES * YB * 4, S], [YB * 4, NCORES], [4, NSLICE]],
            )
            nc.sync.dma_start(
                ysb[:].rearrange("s (c j) -> s c j", c=NCORES), ysrc
            )
            yh = constp.tile([1, N], T)
            ysrc_h = bass.AP(
                yf.tensor, (S - 1) * NCORES * YB * 4,
                [[0, 1], [YB * 4, NCORES], [4, NSLICE]],
            )
            nc.sync.dma_start(yh[:].rearrange("s (c j) -> s c j", c=NCORES), ysrc_h)
            frt = constp.tile([S, N], T)
            nc.sync.dma_start(frt[:], d_fr[:])
            resid = constp.tile([S - 1, N], T)
            nc.vector.tensor_tensor(
                resid[:], frt[: S - 1, :], ysb[: S - 1, :], op=Alu.subtract
            )
            nc.vector.tensor_tensor(resid[:], resid[:], resid[:], op=Alu.mult)
            ones = constp.tile([S - 1, 1], T)
            nc.vector.memset(ones[:], 1.0 / (S - 1))
            var = constp.tile([1, N], T)
            for b in range(10):
                pv = psum.tile([1, 512], T, space="PSUM", tag="pv")
                nc.tensor.matmul(
                    pv[:, :500], ones[:], resid[:, b * 500 : (b + 1) * 500],
                    start=True, stop=True,
                )
                nc.vector.tensor_copy(var[:, b * 500 : (b + 1) * 500], pv[:, :500])
            score = constp.tile([1, N], T)
            nc.vector.tensor_scalar(
                score[:], var[:], aux_t[:, off["gamma"] : off["gamma"] + 1],
                None, op0=Alu.mult,
            )
            nc.vector.tensor_tensor(score[:], yh[:], score[:], op=Alu.subtract)
            nc.scalar.activation(score[:], score[:], Act.Exp)
            ssum = smallp.tile([1, 1], T, tag="ssum")
            nc.vector.reduce_sum(ssum[:], score[:], axis=mybir.AxisListType.X)
            nc.vector.reciprocal(ssum[:], ssum[:])
            nc.vector.tensor_scalar(score[:], score[:], ssum[:], None, op0=Alu.mult)
            nc.vector.tensor_scalar(score[:], score[:], 0.25, None, op0=Alu.min)
            nc.vector.reduce_sum(ssum[:], score[:], axis=mybir.AxisListType.X)
            nc.vector.reciprocal(ssum[:], ssum[:])
            nc.vector.tensor_scalar(score[:], score[:], ssum[:], None, op0=Alu.mult)
            nc.sync.dma_start(d_w[:], score[:])

    tile_patch.walrus_compat(nc)

    kq1 = np.concatenate(
        [np.asarray(inputs["k1"], np.float32), np.asarray(inputs["q1"], np.float32),
         np.asarray(inputs["e1"], np.float32),
         np.asarray(inputs["lew1"], np.float32).reshape(O1, 1)], axis=1)
    kq2 = np.concatenate(
        [np.asarray(inputs["k2"], np.float32), np.asarray(inputs["q2"], np.float32),
         np.asarray(inputs["e2"], np.float32),
         np.asarray(inputs["lew2"], np.float32).reshape(O2, 1)], axis=1)

    shared = dict(
        fr=fr, wall1=wall1, wall2=wall2, w1T=w1T, w2T=w2T, kq1=kq1, kq2=kq2,
        aux=aux,
    )
    in_maps = []
    for c in range(NCORES):
        idx, eav, etf = repack(percore[c])
        m = dict(shared)
        m["xT"] = np.ascontiguousarray(xT[:, :, c * NSLICE : (c + 1) * NSLICE])
        m["idx"] = idx
        m["ea"] = eav
        m["etf"] = etf
        m["sqi"] = percore[c]["sqi"]
        m["ysi"] = percore[c]["ysi"]
        in_maps.append(m)

    global last_nc, last_in_maps
    last_nc, last_in_maps = nc, in_maps
    res = run_bass_kernel_spmd(nc, in_maps, list(range(NCORES)))
    last_results = res
    return res.results[0]["w"].reshape(N).astype(np.float32)


def kernel(x, edge_attr, future_ret, w1, q1, k1, e1, lew1, b1,
           w2, q2, k2, e2, lew2, b2,
           bn1_w, bn1_b, bn1_m, bn1_v, bn2_w, bn2_b, bn2_m, bn2_v,
           lin_w, lin_b, gamma, edge_index, edge_type):
    inputs = dict(
        x=x, edge_attr=edge_attr, future_ret=future_ret, w1=w1, q1=q1, k1=k1,
        e1=e1, lew1=lew1, b1=b1, w2=w2, q2=q2, k2=k2, e2=e2, lew2=lew2, b2=b2,
        bn1_w=bn1_w, bn1_b=bn1_b, bn1_m=bn1_m, bn1_v=bn1_v, bn2_w=bn2_w,
        bn2_b=bn2_b, bn2_m=bn2_m, bn2_v=bn2_v, lin_w=lin_w, lin_b=lin_b,
        gamma=gamma, edge_index=edge_index, edge_type=edge_type,
    )
    return _build_and_run(inputs)
